# revision 22
# baseline (speedup 1.0000x reference)
import os
import sys

import numpy as np

if "/opt/trn_rl_repo" not in sys.path:
    sys.path.insert(0, "/opt/trn_rl_repo")

B, S, HID, H = 1, 2048, 2048, 16
NOPE = ROPE = 64
D = NOPE + ROPE
V = 64
R = 512
SW = 512
NCORES = 8
Q = S // NCORES
KH = Q + SW
NJB = KH // 128
NIT = Q // 128
NSLOT = 5
SCALE = float(D) ** -0.5
DEBUG = bool(int(os.environ.get("BASSDBG", "0")))

JB_OF_SLOT = ((0, 1, 2, 3, 4), (5, 1, 2, 3, 4))

_CACHE = {}


def _build_program():
    import concourse.bass as bass
    import concourse.mybir as mybir
    from concourse import tile
    from contextlib import ExitStack

    f32 = mybir.dt.float32
    f16 = mybir.dt.float16
    AF = mybir.ActivationFunctionType
    OP = mybir.AluOpType

    nc = bass.Bass()

    hsT_d = nc.dram_tensor("hsT", [128, 16 * KH], f16, kind="ExternalInput")
    wqT_d = nc.dram_tensor("wqT", [128, 8 * 16 * 256], f16, kind="ExternalInput")
    wkvaT_d = nc.dram_tensor("wkvaT", [128, 16 * (R + ROPE)], f16, kind="ExternalInput")
    wkc_d = nc.dram_tensor("wkc", [128, 4 * H * NOPE], f16, kind="ExternalInput")
    wvc_d = nc.dram_tensor("wvc", [128, 4 * H * V], f16, kind="ExternalInput")
    woT_d = nc.dram_tensor("woT", [128, 8 * HID], f16, kind="ExternalInput")
    bq_d = nc.dram_tensor("bq", [128, 16], f32, kind="ExternalInput")
    bkva_d = nc.dram_tensor("bkva", [128, 5], f32, kind="ExternalInput")
    bo_d = nc.dram_tensor("bo", [128, HID], f16, kind="ExternalInput")
    tqc_d = nc.dram_tensor("trigq_cos", [128, Q], f16, kind="ExternalInput")
    tqs_d = nc.dram_tensor("trigq_sin", [128, Q], f16, kind="ExternalInput")
    tk_d = nc.dram_tensor("trigk", [64, 2, KH], f16, kind="ExternalInput")
    mUL_d = nc.dram_tensor("maskUL", [128, 2, 128], f16, kind="ExternalInput")
    jvalid_d = nc.dram_tensor("jvalid", [128, NJB], f16, kind="ExternalInput")
    esink_d = nc.dram_tensor("esink", [128, H], f32, kind="ExternalInput")
    out_d = nc.dram_tensor("out", [Q, HID], f32, kind="ExternalOutput")

    dbg = {}
    if DEBUG:
        dbg["lat"] = nc.dram_tensor("dbg_lat", [128, 4, KH], f16, kind="ExternalOutput")
        dbg["lat4"] = nc.dram_tensor("dbg_lat4", [64, KH], f16, kind="ExternalOutput")
        dbg["q"] = nc.dram_tensor("dbg_q", [128, H, Q], f16, kind="ExternalOutput")
        dbg["kf"] = nc.dram_tensor("dbg_kf", [128, H, KH], f16, kind="ExternalOutput")
        dbg["v"] = nc.dram_tensor("dbg_v", [128, NJB, H * 65], f16, kind="ExternalOutput")
        dbg["pr"] = nc.dram_tensor("dbg_pr", [128, H, NSLOT, 256], f16, kind="ExternalOutput")
        dbg["oatq"] = nc.dram_tensor("dbg_oatq", [128, NIT, H * V], f16, kind="ExternalOutput")
        dbg["oat"] = nc.dram_tensor("dbg_oat", [128, 8, Q], f16, kind="ExternalOutput")

    with tile.TileContext(nc) as tc, ExitStack() as ctx:
        const = ctx.enter_context(tc.tile_pool(name="const", bufs=1))

        hs = const.tile([128, 16, KH], f16)
        wkc = const.tile([128, 4, H * NOPE], f16)
        wvc = const.tile([128, 4, H * V], f16)
        wo_sb = const.tile([128, 8, HID], f16)
        bq_sb = const.tile([128, 16], f32)
        bkva_sb = const.tile([128, 5], f32)
        bo_sb = const.tile([128, HID], f16)
        tqc = const.tile([128, Q], f16)
        tqs = const.tile([128, Q], f16)
        tk = const.tile([64, 2, KH], f16)
        mUL = const.tile([128, 2, 128], f16)
        jvalid_sb = const.tile([128, NJB], f16)
        esink_sb = const.tile([128, H], f32)

        qT = const.tile([128, H, Q], f16)
        latbf = const.tile([128, 4, KH], f16)
        lat4 = const.tile([64, KH], f16)
        kf = const.tile([128, H, KH], f16)
        v65 = const.tile([128, NJB, H * 65], f16)
        oatq = const.tile([128, NIT, H * V], f16)
        oat = const.tile([128, 8, Q], f16)
        rot = const.tile([128, KH], f16)

        def bc(ap, n):
            return bass.AP(ap.tensor, ap.offset, [ap.ap[0], [0, n], ap.ap[1]])

        def bc_in(ap, n):
            return bass.AP(ap.tensor, ap.offset, [ap.ap[0], ap.ap[1], [0, n]])

        wkva = const.tile([128, 16, R + ROPE], f16)
        wqp = ctx.enter_context(tc.tile_pool(name="wqch", bufs=5))

        for c in range(2):
            nc.scalar.dma_start(
                wkva[:, 2 * c : 2 * c + 2, :],
                wkvaT_d[:, 2 * c * 576 : (2 * c + 2) * 576],
            )
        nc.scalar.dma_start(bkva_sb[:], bkva_d[:])
        nc.scalar.dma_start(jvalid_sb[:], jvalid_d[:])
        for c in range(2, 8):
            nc.scalar.dma_start(
                wkva[:, 2 * c : 2 * c + 2, :],
                wkvaT_d[:, 2 * c * 576 : (2 * c + 2) * 576],
            )
        nc.scalar.dma_start(tk[:], tk_d[:])
        nc.scalar.dma_start(mUL[:], mUL_d[:])
        nc.scalar.dma_start(esink_sb[:], esink_d[:])
        nc.scalar.dma_start(bq_sb[:], bq_d[:])
        nc.scalar.dma_start(tqc[:], tqc_d[:])
        nc.scalar.dma_start(tqs[:], tqs_d[:])
        for c in range(8):
            nc.sync.dma_start(
                hs[:, 2 * c : 2 * c + 2, :], hsT_d[:, 2 * c * KH : (2 * c + 2) * KH]
            )
        for c in range(2):
            nc.sync.dma_start(
                wkc[:, 2 * c : 2 * c + 2, :], wkc_d[:, 2 * c * 1024 : (2 * c + 2) * 1024]
            )
        for c in range(2):
            nc.sync.dma_start(
                wvc[:, 2 * c : 2 * c + 2, :], wvc_d[:, 2 * c * 1024 : (2 * c + 2) * 1024]
            )
        wq_ch = []
        for p in range(8):
            t = wqp.tile([128, 16, 256], f16, tag="wq")
            eng = nc.sync if p % 2 == 0 else nc.scalar
            eng.dma_start(t[:], wqT_d[:, p * 4096 : (p + 1) * 4096])
            wq_ch.append(t)
        for c in range(8):
            nc.sync.dma_start(wo_sb[:, c, :], woT_d[:, c * HID : (c + 1) * HID])
        nc.scalar.dma_start(bo_sb[:], bo_d[:])

        jvb = bc_in(jvalid_sb[:], 128)
        jvb64 = bc_in(jvalid_sb[0:64, :], 128)

        with tc.tile_pool(name="pslat", bufs=1, space="PSUM") as pslatp:
            pslat = [
                pslatp.tile([128, KH], f32, tag=f"pslat{m}", name=f"pslat{m}")
                for m in range(4)
            ]
            for k in range(16):
                for m in range(4):
                    for n0, n1 in ((0, 512), (512, KH)):
                        nc.tensor.matmul(
                            pslat[m][:, n0:n1],
                            lhsT=wkva[:, k, m * 128 : (m + 1) * 128],
                            rhs=hs[:, k, n0:n1],
                            start=(k == 0),
                            stop=(k == 15),
                        )
            for m in range(4):
                nc.vector.scalar_tensor_tensor(
                    latbf[:, m, :], pslat[m][:], bkva_sb[:, m : m + 1],
                    jvb, OP.add, OP.mult,
                )
            ps4 = pslatp.tile([64, KH], f32, tag="pslat0")
            for k in range(16):
                for n0, n1 in ((0, 512), (512, KH)):
                    nc.tensor.matmul(
                        ps4[:, n0:n1],
                        lhsT=wkva[:, k, 512:576],
                        rhs=hs[:, k, n0:n1],
                        start=(k == 0),
                        stop=(k == 15),
                    )
            nc.vector.scalar_tensor_tensor(
                lat4[:], ps4[:], bkva_sb[0:64, 4:5], jvb64, OP.add, OP.mult,
            )

        rotk = rot[0:64, :]
        nc.vector.tensor_copy(rotk[0:32, :], lat4[32:64, :])
        nc.vector.tensor_copy(rotk[32:64, :], lat4[0:32, :])
        nc.vector.tensor_mul(lat4[0:32, :], lat4[0:32, :], tk[0:32, 0, :])
        nc.vector.tensor_mul(rotk[0:32, :], rotk[0:32, :], tk[0:32, 1, :])
        nc.vector.tensor_sub(lat4[0:32, :], lat4[0:32, :], rotk[0:32, :])
        nc.vector.tensor_mul(lat4[32:64, :], lat4[32:64, :], tk[32:64, 0, :])
        nc.vector.tensor_mul(rotk[32:64, :], rotk[32:64, :], tk[32:64, 1, :])
        nc.vector.tensor_add(lat4[32:64, :], lat4[32:64, :], rotk[32:64, :])
        for h in range(H):
            nc.sync.dma_start(kf[64:128, h, :], lat4[:])

        knvp_ctx = ExitStack()
        knp = knvp_ctx.enter_context(tc.tile_pool(name="pskn", bufs=2, space="PSUM"))
        psvp = knvp_ctx.enter_context(tc.tile_pool(name="psv", bufs=2, space="PSUM"))

        def emit_knope(m):
            ps = knp.tile([128, KH], f32, tag="pskn")
            for k in range(4):
                for n0, n1 in ((0, 512), (512, KH)):
                    nc.tensor.matmul(
                        ps[:, n0:n1],
                        lhsT=wkc[:, k, m * 128 : (m + 1) * 128],
                        rhs=latbf[:, k, n0:n1],
                        start=(k == 0),
                        stop=(k == 3),
                    )
            nc.vector.tensor_copy(kf[0:64, 2 * m, :], ps[0:64, :])
            nc.vector.tensor_copy(kf[0:64, 2 * m + 1, :], ps[64:128, :])

        def emit_v(jb):
            vview = v65[:, jb, :].rearrange("p (h d) -> p h d", d=65)
            for half in range(2):
                ps = psvp.tile([128, 512], f32, tag="psv")
                n0 = half * 512
                for k in range(4):
                    nc.tensor.matmul(
                        ps[:],
                        lhsT=latbf[:, k, jb * 128 : (jb + 1) * 128],
                        rhs=wvc[:, k, n0 : n0 + 512],
                        start=(k == 0),
                        stop=(k == 3),
                    )
                ps_view = ps[:].rearrange("p (h d) -> p h d", d=V)
                if half == 0:
                    nc.scalar.copy(vview[:, 0:8, 0:V], ps_view)
                else:
                    nc.vector.tensor_copy(vview[:, 8:16, 0:V], ps_view)
            nc.scalar.copy(vview[:, :, V : V + 1], bc(jvalid_sb[:, jb : jb + 1], H))

        emit_knope(0)
        emit_knope(1)
        emit_v(0)
        emit_knope(2)
        emit_v(1)
        emit_knope(3)
        emit_v(2)
        emit_knope(4)
        emit_v(3)
        emit_knope(5)
        emit_v(4)
        emit_knope(6)
        emit_v(5)
        emit_knope(7)
        knvp_ctx.close()

        att_ctx = ExitStack()
        attp = att_ctx.enter_context(tc.tile_pool(name="att_sbuf", bufs=3))
        attps = att_ctx.enter_context(tc.tile_pool(name="att_psum", bufs=1, space="PSUM"))
        statp = att_ctx.enter_context(tc.tile_pool(name="stat", bufs=4))
        psqp = att_ctx.enter_context(tc.tile_pool(name="psq", bufs=1, space="PSUM"))

        pr_tiles = {}

        def emit_qpair(p):
            wqt = wq_ch[p]
            psq = [
                psqp.tile([128, Q], f32, tag=f"psq{m}", name=f"psq{p}_{m}")
                for m in range(2)
            ]
            for k in range(16):
                for m in range(2):
                    nc.tensor.matmul(
                        psq[m][:],
                        lhsT=wqt[:, k, m * 128 : (m + 1) * 128],
                        rhs=hs[:, k, SW:KH],
                        start=(k == 0),
                        stop=(k == 15),
                    )
            for m in range(2):
                hh = 2 * p + m
                nc.scalar.activation(
                    qT[:, hh, :], psq[m][:], AF.Identity,
                    bias=bq_sb[:, hh : hh + 1], scale=1.0,
                )
            rotq = rot[:, 0:512].rearrange("p (a b) -> p a b", b=256)
            hs_ = slice(2 * p, 2 * p + 2)
            nc.vector.tensor_copy(rotq[64:96, :, :], qT[96:128, hs_, :])
            nc.vector.tensor_copy(rotq[96:128, :, :], qT[64:96, hs_, :])
            nc.vector.tensor_mul(qT[64:96, hs_, :], qT[64:96, hs_, :], bc(tqc[64:96, :], 2))
            nc.vector.tensor_mul(rotq[64:96, :, :], rotq[64:96, :, :], bc(tqs[64:96, :], 2))
            nc.vector.tensor_sub(qT[64:96, hs_, :], qT[64:96, hs_, :], rotq[64:96, :, :])
            nc.vector.tensor_mul(qT[96:128, hs_, :], qT[96:128, hs_, :], bc(tqc[96:128, :], 2))
            nc.vector.tensor_mul(rotq[96:128, :, :], rotq[96:128, :, :], bc(tqs[96:128, :], 2))
            nc.vector.tensor_add(qT[96:128, hs_, :], qT[96:128, hs_, :], rotq[96:128, :, :])

        def emit_scores(h):
            ps_s = attps.tile([128, NSLOT, 256], f32, tag="ps_s")
            nc.tensor.matmul(
                ps_s[:, 0, 0:128],
                lhsT=kf[:, h, 0:128],
                rhs=qT[:, h, 0:128],
                start=True, stop=True,
            )
            nc.tensor.matmul(
                ps_s[:, 0, 128:256],
                lhsT=kf[:, h, 640:768],
                rhs=qT[:, h, 128:256],
                start=True, stop=True,
            )
            for s in range(1, 5):
                nc.tensor.matmul(
                    ps_s[:, s, :],
                    lhsT=kf[:, h, s * 128 : (s + 1) * 128],
                    rhs=qT[:, h, :],
                    start=True, stop=True,
                )
            pr = attp.tile([128, NSLOT, 256], f16, tag="pr")
            nc.scalar.activation(pr[:], ps_s[:], AF.Exp, bias=0.0, scale=SCALE)
            nc.vector.tensor_mul(
                pr[:, 0, :].rearrange("p (a b) -> p a b", b=128),
                pr[:, 0, :].rearrange("p (a b) -> p a b", b=128), mUL[:],
            )
            nc.vector.tensor_mul(pr[:, 1, 128:256], pr[:, 1, 128:256], mUL[:, 0, :])
            nc.vector.tensor_mul(pr[:, 4, 0:128], pr[:, 4, 0:128], mUL[:, 1, :])
            pr_tiles[h] = pr
            if DEBUG:
                nc.sync.dma_start(dbg["pr"][:, h, :, :], pr[:])

        def emit_pv_pair(m):
            for h in (2 * m, 2 * m + 1):
                pr = pr_tiles.pop(h)
                ps_o = attps.tile([128, 130], f32, tag="ps_o", bufs=2)
                for it in range(NIT):
                    for n, jb in enumerate(JB_OF_SLOT[it]):
                        nc.tensor.matmul(
                            ps_o[:, it * 65 : it * 65 + 65],
                            lhsT=pr[:, n, it * 128 : (it + 1) * 128],
                            rhs=v65[:, jb, h * 65 : (h + 1) * 65],
                            start=(n == 0),
                            stop=(n == 4),
                        )
                dsc = statp.tile([128, 2], f32, tag="dsc")
                nc.vector.tensor_scalar(
                    dsc[:, 0:1], ps_o[:, 64:65], esink_sb[:, h : h + 1], None, OP.add
                )
                nc.vector.tensor_scalar(
                    dsc[:, 1:2], ps_o[:, 129:130], esink_sb[:, h : h + 1], None, OP.add
                )
                rcp = statp.tile([128, 2], f32, tag="rcp")
                nc.vector.reciprocal(rcp[:], dsc[:])
                for it in range(NIT):
                    nc.scalar.activation(
                        oatq[:, it, h * V : (h + 1) * V],
                        ps_o[:, it * 65 : it * 65 + V], AF.Identity,
                        bias=0.0, scale=rcp[:, it : it + 1],
                    )
            for it in range(NIT):
                nc.scalar.dma_start(
                    oat[:, m, it * 128 : (it + 1) * 128],
                    oatq[:, it, m * 128 : (m + 1) * 128],
                    transpose=True,
                )

        emit_qpair(0)
        emit_qpair(1)
        emit_scores(0)
        emit_qpair(2)
        emit_scores(1)
        emit_scores(2)
        emit_pv_pair(0)
        emit_qpair(3)
        emit_scores(3)
        emit_scores(4)
        emit_pv_pair(1)
        emit_qpair(4)
        emit_scores(5)
        emit_scores(6)
        emit_pv_pair(2)
        emit_qpair(5)
        emit_scores(7)
        emit_scores(8)
        emit_pv_pair(3)
        emit_qpair(6)
        emit_scores(9)
        emit_scores(10)
        emit_pv_pair(4)
        emit_qpair(7)
        emit_scores(11)
        emit_scores(12)
        emit_pv_pair(5)
        emit_scores(13)
        emit_scores(14)
        emit_pv_pair(6)
        emit_scores(15)
        emit_pv_pair(7)
        att_ctx.close()

        with tc.tile_pool(name="psf", bufs=2, space="PSUM") as psfp, tc.tile_pool(
            name="outp", bufs=2
        ) as outp:
            for it in range(NIT):
                for n in range(4):
                    psf = psfp.tile([128, 512], f32, tag="psf")
                    for k in range(8):
                        nc.tensor.matmul(
                            psf[:],
                            lhsT=oat[:, k, it * 128 : (it + 1) * 128],
                            rhs=wo_sb[:, k, n * 512 : (n + 1) * 512],
                            start=(k == 0),
                            stop=(k == 7),
                        )
                    ob = outp.tile([128, 512], f32, tag="ob")
                    nc.vector.tensor_add(
                        ob[:], psf[:], bo_sb[:, n * 512 : (n + 1) * 512]
                    )
                    nc.sync.dma_start(
                        out_d[it * 128 : (it + 1) * 128, n * 512 : (n + 1) * 512],
                        ob[:],
                    )

        if DEBUG:
            nc.sync.dma_start(dbg["lat"][:], latbf[:])
            nc.sync.dma_start(dbg["lat4"][:], lat4[:])
            nc.sync.dma_start(dbg["q"][:], qT[:])
            nc.sync.dma_start(dbg["kf"][:], kf[:])
            nc.sync.dma_start(dbg["v"][:], v65[:])
            nc.sync.dma_start(dbg["oatq"][:], oatq[:])
            nc.sync.dma_start(dbg["oat"][:], oat[:])

    if not bool(int(os.environ.get("BASSNOSPLIT", "0"))):
        _split_multi_waits(nc, mybir)
    nc.finalize()
    return nc


def _split_multi_waits(nc, mybir):
    seq_ok = (mybir.InstEventSemaphore,)
    n = 0
    for fn in nc.m.functions:
        for blk in fn.blocks:
            out = []
            for inst in blk.instructions:
                si = inst.sync_info
                if si is not None and len(si.on_wait) > 1 and not isinstance(inst, seq_ok):
                    if isinstance(inst, mybir.InstDMACopy) and inst.engine not in (
                        mybir.EngineType.SP,
                        mybir.EngineType.Activation,
                    ):
                        raise AssertionError(
                            f"DMA {inst.name} has {len(si.on_wait)} waits; "
                            "restructure so DMAs carry at most one"
                        )
                    for w in si.on_wait[:-1]:
                        n += 1
                        out.append(
                            mybir.InstEventSemaphore(
                                name=f"I-wsplit-{n}",
                                engine=inst.engine,
                                ins=[],
                                outs=[],
                                sync_info=mybir.SyncInfo(on_wait=[w], on_update=[]),
                            )
                        )
                    inst.sync_info = mybir.SyncInfo(
                        on_wait=[si.on_wait[-1]], on_update=si.on_update
                    )
                out.append(inst)
            blk.instructions = out
    return n


def prep_inputs(
    hidden_states, cos, sin, Wq, bq, Wo, bo, Wkva, bkva, w_kc, w_vc, sinks
):
    f16 = np.float16
    hs = np.asarray(hidden_states, np.float32)[0]
    cos = np.asarray(cos, np.float32)[0]
    sin = np.asarray(sin, np.float32)[0]

    def sbuf_pack(a):
        kf_, f_ = a.shape
        k_ = kf_ // 128
        return np.ascontiguousarray(
            a.reshape(k_, 128, f_).transpose(1, 0, 2).reshape(128, k_ * f_)
        )

    wq_kmaj = sbuf_pack(np.asarray(Wq, np.float32).T)
    wqT = np.ascontiguousarray(
        wq_kmaj.reshape(128, 16, 8, 256).transpose(0, 2, 1, 3).reshape(128, -1)
    ).astype(f16)
    wkvaT = sbuf_pack(np.asarray(Wkva, np.float32).T).astype(f16)
    wkc_p = sbuf_pack(
        np.asarray(w_kc, np.float32).transpose(2, 0, 1).reshape(R, H * NOPE)
    ).astype(f16)
    wvc_p = sbuf_pack(
        np.asarray(w_vc, np.float32).transpose(1, 0, 2).reshape(R, H * V)
    ).astype(f16)
    woT = sbuf_pack(np.asarray(Wo, np.float32).T).astype(f16)

    bq_t = np.ascontiguousarray(np.asarray(bq, np.float32).reshape(16, 128).T)
    bkva_pad = np.zeros(640, np.float32)
    bkva_pad[: R + ROPE] = np.asarray(bkva, np.float32)
    bkva_t = np.ascontiguousarray(bkva_pad.reshape(5, 128).T)
    bo_b = np.ascontiguousarray(
        np.broadcast_to(np.asarray(bo, np.float32), (128, HID))
    ).astype(f16)
    esink_b = np.ascontiguousarray(
        np.broadcast_to(np.exp(np.asarray(sinks, np.float32))[None, :], (128, H))
    )
    pp = np.arange(128)[:, None]
    ii = np.arange(128)[None, :]
    mUL = np.zeros((128, 2, 128), np.float32)
    mUL[:, 0, :] = (ii < pp).astype(np.float32)
    mUL[:, 1, :] = (pp <= ii).astype(np.float32)

    hs_pad = np.zeros((SW + S, HID), np.float32)
    hs_pad[SW:] = hs

    shared = dict(
        wqT=wqT, wkvaT=wkvaT, wkc=wkc_p, wvc=wvc_p, woT=woT,
        bq=bq_t, bkva=bkva_t, bo=bo_b, esink=esink_b,
        maskUL=mUL.astype(f16),
    )

    in_maps = []
    for c in range(NCORES):
        g0 = c * Q
        hsT_c = sbuf_pack(np.ascontiguousarray(hs_pad[g0 : g0 + KH].T)).astype(f16)

        cq = cos[g0 : g0 + Q]
        sq = sin[g0 : g0 + Q]
        tqc = np.zeros((128, Q), np.float32)
        tqs = np.zeros((128, Q), np.float32)
        tqc[64:96] = cq[:, 0:32].T
        tqc[96:128] = cq[:, 32:64].T
        tqs[64:96] = sq[:, 0:32].T
        tqs[96:128] = sq[:, 32:64].T

        kpos = np.clip(np.arange(g0 - SW, g0 + Q), 0, None)
        ck = cos[kpos]
        sk = sin[kpos]
        tkk = np.zeros((64, 2, KH), np.float32)
        tkk[0:32, 0] = ck[:, 0:32].T
        tkk[32:64, 0] = ck[:, 32:64].T
        tkk[0:32, 1] = sk[:, 0:32].T
        tkk[32:64, 1] = sk[:, 32:64].T

        jg = (g0 - SW) + np.arange(KH)
        jvalid = (jg.reshape(NJB, 128).T >= 0).astype(np.float32)

        in_maps.append(
            dict(
                shared,
                hsT=hsT_c,
                trigq_cos=tqc.astype(f16),
                trigq_sin=tqs.astype(f16),
                trigk=tkk.astype(f16),
                jvalid=jvalid.astype(f16),
            )
        )
    return in_maps


def get_program():
    if "nc" not in _CACHE:
        _CACHE["nc"] = _build_program()
    return _CACHE["nc"]


def run(in_maps, **kw):
    from concourse.bass_utils import run_bass_kernel_spmd

    nc = get_program()
    return run_bass_kernel_spmd(nc, in_maps, list(range(NCORES)), **kw)


def kernel(**inputs):
    in_maps = prep_inputs(**inputs)
    res = run(in_maps)
    out = np.concatenate([res.results[c]["out"] for c in range(NCORES)], axis=0)
    return out.reshape(B, S, HID).astype(np.float32)


# revision 23
# speedup vs baseline: 1.0085x; 1.0085x over previous
import os
import sys

import numpy as np

if "/opt/trn_rl_repo" not in sys.path:
    sys.path.insert(0, "/opt/trn_rl_repo")

B, S, HID, H = 1, 2048, 2048, 16
NOPE = ROPE = 64
D = NOPE + ROPE
V = 64
R = 512
SW = 512
NCORES = 8
Q = S // NCORES
KH = Q + SW
NJB = KH // 128
NIT = Q // 128
NSLOT = 5
SCALE = float(D) ** -0.5
DEBUG = bool(int(os.environ.get("BASSDBG", "0")))

JB_OF_SLOT = ((0, 1, 2, 3, 4), (5, 1, 2, 3, 4))

_CACHE = {}


def _build_program():
    import concourse.bass as bass
    import concourse.mybir as mybir
    from concourse import tile
    from contextlib import ExitStack

    f32 = mybir.dt.float32
    f16 = mybir.dt.float16
    AF = mybir.ActivationFunctionType
    OP = mybir.AluOpType

    nc = bass.Bass()

    hsT_d = nc.dram_tensor("hsT", [128, 16 * KH], f16, kind="ExternalInput")
    wqT_d = nc.dram_tensor("wqT", [128, 8 * 16 * 256], f16, kind="ExternalInput")
    wkvaT_d = nc.dram_tensor("wkvaT", [128, 16 * (R + ROPE)], f16, kind="ExternalInput")
    wkc_d = nc.dram_tensor("wkc", [128, 4 * H * NOPE], f16, kind="ExternalInput")
    wvc_d = nc.dram_tensor("wvc", [128, 4 * H * V], f16, kind="ExternalInput")
    woT_d = nc.dram_tensor("woT", [128, 8 * HID], f16, kind="ExternalInput")
    bq_d = nc.dram_tensor("bq", [128, 16], f32, kind="ExternalInput")
    bkva_d = nc.dram_tensor("bkva", [128, 5], f32, kind="ExternalInput")
    bo_d = nc.dram_tensor("bo", [128, HID], f16, kind="ExternalInput")
    tqc_d = nc.dram_tensor("trigq_cos", [128, Q], f16, kind="ExternalInput")
    tqs_d = nc.dram_tensor("trigq_sin", [128, Q], f16, kind="ExternalInput")
    tk_d = nc.dram_tensor("trigk", [64, 2, KH], f16, kind="ExternalInput")
    mUL_d = nc.dram_tensor("maskUL", [128, 2, 128], f16, kind="ExternalInput")
    jvalid_d = nc.dram_tensor("jvalid", [128, NJB], f16, kind="ExternalInput")
    esink_d = nc.dram_tensor("esink", [128, H], f32, kind="ExternalInput")
    out_d = nc.dram_tensor("out", [Q, HID], f32, kind="ExternalOutput")

    dbg = {}
    if DEBUG:
        dbg["lat"] = nc.dram_tensor("dbg_lat", [128, 4, KH], f16, kind="ExternalOutput")
        dbg["lat4"] = nc.dram_tensor("dbg_lat4", [64, KH], f16, kind="ExternalOutput")
        dbg["q"] = nc.dram_tensor("dbg_q", [128, H, Q], f16, kind="ExternalOutput")
        dbg["kf"] = nc.dram_tensor("dbg_kf", [128, H, KH], f16, kind="ExternalOutput")
        dbg["v"] = nc.dram_tensor("dbg_v", [128, NJB, H * 65], f16, kind="ExternalOutput")
        dbg["pr"] = nc.dram_tensor("dbg_pr", [128, H, NSLOT, 256], f16, kind="ExternalOutput")
        dbg["oatq"] = nc.dram_tensor("dbg_oatq", [128, NIT, H * V], f16, kind="ExternalOutput")
        dbg["oat"] = nc.dram_tensor("dbg_oat", [128, 8, Q], f16, kind="ExternalOutput")

    with tile.TileContext(nc) as tc, ExitStack() as ctx:
        const = ctx.enter_context(tc.tile_pool(name="const", bufs=1))

        hs = const.tile([128, 16, KH], f16)
        wkc = const.tile([128, 4, H * NOPE], f16)
        wvc = const.tile([128, 4, H * V], f16)
        wo_sb = const.tile([128, 8, HID], f16)
        bq_sb = const.tile([128, 16], f32)
        bkva_sb = const.tile([128, 5], f32)
        bo_sb = const.tile([128, HID], f16)
        tqc = const.tile([128, Q], f16)
        tqs = const.tile([128, Q], f16)
        tk = const.tile([64, 2, KH], f16)
        mUL = const.tile([128, 2, 128], f16)
        jvalid_sb = const.tile([128, NJB], f16)
        esink_sb = const.tile([128, H], f32)

        qT = const.tile([128, H, Q], f16)
        latbf = const.tile([128, 4, KH], f16)
        lat4 = const.tile([64, KH], f16)
        kf = const.tile([128, H, KH], f16)
        v65 = const.tile([128, NJB, H * 65], f16)
        oatq = const.tile([128, NIT, H * V], f16)
        oat = const.tile([128, 8, Q], f16)
        rot = const.tile([128, KH], f16)

        def bc(ap, n):
            return bass.AP(ap.tensor, ap.offset, [ap.ap[0], [0, n], ap.ap[1]])

        def bc_in(ap, n):
            return bass.AP(ap.tensor, ap.offset, [ap.ap[0], ap.ap[1], [0, n]])

        wkva = const.tile([128, 16, R + ROPE], f16)
        wqp = ctx.enter_context(tc.tile_pool(name="wqch", bufs=5))

        for c in range(2):
            nc.scalar.dma_start(
                wkva[:, 2 * c : 2 * c + 2, :],
                wkvaT_d[:, 2 * c * 576 : (2 * c + 2) * 576],
            )
        nc.scalar.dma_start(bkva_sb[:], bkva_d[:])
        nc.scalar.dma_start(jvalid_sb[:], jvalid_d[:])
        for c in range(2, 8):
            nc.scalar.dma_start(
                wkva[:, 2 * c : 2 * c + 2, :],
                wkvaT_d[:, 2 * c * 576 : (2 * c + 2) * 576],
            )
        nc.scalar.dma_start(tk[:], tk_d[:])
        nc.scalar.dma_start(mUL[:], mUL_d[:])
        nc.scalar.dma_start(esink_sb[:], esink_d[:])
        nc.scalar.dma_start(bq_sb[:], bq_d[:])
        nc.scalar.dma_start(tqc[:], tqc_d[:])
        nc.scalar.dma_start(tqs[:], tqs_d[:])
        for c in range(8):
            nc.sync.dma_start(
                hs[:, 2 * c : 2 * c + 2, :], hsT_d[:, 2 * c * KH : (2 * c + 2) * KH]
            )
        for c in range(2):
            nc.sync.dma_start(
                wkc[:, 2 * c : 2 * c + 2, :], wkc_d[:, 2 * c * 1024 : (2 * c + 2) * 1024]
            )
        for c in range(2):
            nc.sync.dma_start(
                wvc[:, 2 * c : 2 * c + 2, :], wvc_d[:, 2 * c * 1024 : (2 * c + 2) * 1024]
            )
        wq_ch = []
        for p in range(8):
            t = wqp.tile([128, 16, 256], f16, tag="wq")
            eng = nc.sync if p % 2 == 0 else nc.scalar
            eng.dma_start(t[:], wqT_d[:, p * 4096 : (p + 1) * 4096])
            wq_ch.append(t)
        for c in range(8):
            nc.sync.dma_start(wo_sb[:, c, :], woT_d[:, c * HID : (c + 1) * HID])
        nc.scalar.dma_start(bo_sb[:], bo_d[:])

        jvb = bc_in(jvalid_sb[:], 128)
        jvb64 = bc_in(jvalid_sb[0:64, :], 128)

        with tc.tile_pool(name="pslat", bufs=1, space="PSUM") as pslatp:
            pslat = [
                pslatp.tile([128, KH], f32, tag=f"pslat{m}", name=f"pslat{m}")
                for m in range(4)
            ]
            for k in range(16):
                for m in range(4):
                    for n0, n1 in ((0, 512), (512, KH)):
                        nc.tensor.matmul(
                            pslat[m][:, n0:n1],
                            lhsT=wkva[:, k, m * 128 : (m + 1) * 128],
                            rhs=hs[:, k, n0:n1],
                            start=(k == 0),
                            stop=(k == 15),
                        )
            for m in range(4):
                nc.vector.scalar_tensor_tensor(
                    latbf[:, m, :], pslat[m][:], bkva_sb[:, m : m + 1],
                    jvb, OP.add, OP.mult,
                )
            ps4 = pslatp.tile([64, KH], f32, tag="pslat0")
            for k in range(16):
                for n0, n1 in ((0, 512), (512, KH)):
                    nc.tensor.matmul(
                        ps4[:, n0:n1],
                        lhsT=wkva[:, k, 512:576],
                        rhs=hs[:, k, n0:n1],
                        start=(k == 0),
                        stop=(k == 15),
                    )
            nc.vector.scalar_tensor_tensor(
                lat4[:], ps4[:], bkva_sb[0:64, 4:5], jvb64, OP.add, OP.mult,
            )

        rotk = rot[0:64, :]
        nc.vector.tensor_copy(rotk[0:32, :], lat4[32:64, :])
        nc.vector.tensor_copy(rotk[32:64, :], lat4[0:32, :])
        nc.vector.tensor_mul(lat4[0:32, :], lat4[0:32, :], tk[0:32, 0, :])
        nc.vector.tensor_mul(rotk[0:32, :], rotk[0:32, :], tk[0:32, 1, :])
        nc.vector.tensor_sub(lat4[0:32, :], lat4[0:32, :], rotk[0:32, :])
        nc.vector.tensor_mul(lat4[32:64, :], lat4[32:64, :], tk[32:64, 0, :])
        nc.vector.tensor_mul(rotk[32:64, :], rotk[32:64, :], tk[32:64, 1, :])
        nc.vector.tensor_add(lat4[32:64, :], lat4[32:64, :], rotk[32:64, :])
        for h in range(H):
            nc.sync.dma_start(kf[64:128, h, :], lat4[:])

        knvp_ctx = ExitStack()
        knp = knvp_ctx.enter_context(tc.tile_pool(name="pskn", bufs=3, space="PSUM"))
        psvp = knvp_ctx.enter_context(tc.tile_pool(name="psv", bufs=2, space="PSUM"))

        def emit_knope(m):
            ps = knp.tile([128, KH], f32, tag="pskn")
            for k in range(4):
                for n0, n1 in ((0, 512), (512, KH)):
                    nc.tensor.matmul(
                        ps[:, n0:n1],
                        lhsT=wkc[:, k, m * 128 : (m + 1) * 128],
                        rhs=latbf[:, k, n0:n1],
                        start=(k == 0),
                        stop=(k == 3),
                    )
            nc.vector.tensor_copy(kf[0:64, 2 * m, :], ps[0:64, :])
            nc.vector.tensor_copy(kf[0:64, 2 * m + 1, :], ps[64:128, :])

        def emit_v(jb):
            vview = v65[:, jb, :].rearrange("p (h d) -> p h d", d=65)
            for half in range(2):
                ps = psvp.tile([128, 512], f32, tag="psv")
                n0 = half * 512
                for k in range(4):
                    nc.tensor.matmul(
                        ps[:],
                        lhsT=latbf[:, k, jb * 128 : (jb + 1) * 128],
                        rhs=wvc[:, k, n0 : n0 + 512],
                        start=(k == 0),
                        stop=(k == 3),
                    )
                ps_view = ps[:].rearrange("p (h d) -> p h d", d=V)
                if half == 0:
                    nc.scalar.copy(vview[:, 0:8, 0:V], ps_view)
                else:
                    nc.vector.tensor_copy(vview[:, 8:16, 0:V], ps_view)
            nc.scalar.copy(vview[:, :, V : V + 1], bc(jvalid_sb[:, jb : jb + 1], H))

        emit_knope(0)
        emit_knope(1)
        emit_v(0)
        emit_knope(2)
        emit_v(1)
        emit_knope(3)
        emit_v(2)
        emit_knope(4)
        emit_v(3)
        emit_knope(5)
        emit_v(4)
        emit_knope(6)
        emit_v(5)
        emit_knope(7)
        knvp_ctx.close()

        att_ctx = ExitStack()
        attp = att_ctx.enter_context(tc.tile_pool(name="att_sbuf", bufs=3))
        attps = att_ctx.enter_context(tc.tile_pool(name="att_psum", bufs=1, space="PSUM"))
        statp = att_ctx.enter_context(tc.tile_pool(name="stat", bufs=4))
        psqp = att_ctx.enter_context(tc.tile_pool(name="psq", bufs=1, space="PSUM"))

        pr_tiles = {}

        def emit_qpair(p):
            wqt = wq_ch[p]
            psq = [
                psqp.tile([128, Q], f32, tag=f"psq{m}", name=f"psq{p}_{m}")
                for m in range(2)
            ]
            for k in range(16):
                for m in range(2):
                    nc.tensor.matmul(
                        psq[m][:],
                        lhsT=wqt[:, k, m * 128 : (m + 1) * 128],
                        rhs=hs[:, k, SW:KH],
                        start=(k == 0),
                        stop=(k == 15),
                    )
            for m in range(2):
                hh = 2 * p + m
                if p == 7 and m == 1:
                    nc.vector.tensor_scalar(
                        qT[:, hh, :], psq[m][:], bq_sb[:, hh : hh + 1], None, OP.add
                    )
                else:
                    nc.scalar.activation(
                        qT[:, hh, :], psq[m][:], AF.Identity,
                        bias=bq_sb[:, hh : hh + 1], scale=1.0,
                    )
            rotq = rot[:, 0:512].rearrange("p (a b) -> p a b", b=256)
            hs_ = slice(2 * p, 2 * p + 2)
            nc.vector.tensor_copy(rotq[64:96, :, :], qT[96:128, hs_, :])
            nc.vector.tensor_copy(rotq[96:128, :, :], qT[64:96, hs_, :])
            nc.vector.tensor_mul(qT[64:96, hs_, :], qT[64:96, hs_, :], bc(tqc[64:96, :], 2))
            nc.vector.tensor_mul(rotq[64:96, :, :], rotq[64:96, :, :], bc(tqs[64:96, :], 2))
            nc.vector.tensor_sub(qT[64:96, hs_, :], qT[64:96, hs_, :], rotq[64:96, :, :])
            nc.vector.tensor_mul(qT[96:128, hs_, :], qT[96:128, hs_, :], bc(tqc[96:128, :], 2))
            nc.vector.tensor_mul(rotq[96:128, :, :], rotq[96:128, :, :], bc(tqs[96:128, :], 2))
            nc.vector.tensor_add(qT[96:128, hs_, :], qT[96:128, hs_, :], rotq[96:128, :, :])

        def emit_scores(h):
            ps_s = attps.tile([128, NSLOT, 256], f32, tag="ps_s")
            nc.tensor.matmul(
                ps_s[:, 0, 0:128],
                lhsT=kf[:, h, 0:128],
                rhs=qT[:, h, 0:128],
                start=True, stop=True,
            )
            nc.tensor.matmul(
                ps_s[:, 0, 128:256],
                lhsT=kf[:, h, 640:768],
                rhs=qT[:, h, 128:256],
                start=True, stop=True,
            )
            for s in range(1, 5):
                nc.tensor.matmul(
                    ps_s[:, s, :],
                    lhsT=kf[:, h, s * 128 : (s + 1) * 128],
                    rhs=qT[:, h, :],
                    start=True, stop=True,
                )
            pr = attp.tile([128, NSLOT, 256], f16, tag="pr")
            nc.scalar.activation(pr[:], ps_s[:], AF.Exp, bias=0.0, scale=SCALE)
            nc.vector.tensor_mul(
                pr[:, 0, :].rearrange("p (a b) -> p a b", b=128),
                pr[:, 0, :].rearrange("p (a b) -> p a b", b=128), mUL[:],
            )
            nc.vector.tensor_mul(pr[:, 1, 128:256], pr[:, 1, 128:256], mUL[:, 0, :])
            nc.vector.tensor_mul(pr[:, 4, 0:128], pr[:, 4, 0:128], mUL[:, 1, :])
            pr_tiles[h] = pr
            if DEBUG:
                nc.sync.dma_start(dbg["pr"][:, h, :, :], pr[:])

        def emit_pv_pair(m):
            for h in (2 * m, 2 * m + 1):
                pr = pr_tiles.pop(h)
                ps_o = attps.tile([128, 130], f32, tag="ps_o", bufs=3)
                for it in range(NIT):
                    for n, jb in enumerate(JB_OF_SLOT[it]):
                        nc.tensor.matmul(
                            ps_o[:, it * 65 : it * 65 + 65],
                            lhsT=pr[:, n, it * 128 : (it + 1) * 128],
                            rhs=v65[:, jb, h * 65 : (h + 1) * 65],
                            start=(n == 0),
                            stop=(n == 4),
                        )
                dsc = statp.tile([128, 2], f32, tag="dsc")
                nc.vector.tensor_scalar(
                    dsc[:, 0:1], ps_o[:, 64:65], esink_sb[:, h : h + 1], None, OP.add
                )
                nc.vector.tensor_scalar(
                    dsc[:, 1:2], ps_o[:, 129:130], esink_sb[:, h : h + 1], None, OP.add
                )
                rcp = statp.tile([128, 2], f32, tag="rcp")
                nc.vector.reciprocal(rcp[:], dsc[:])
                for it in range(NIT):
                    nc.scalar.activation(
                        oatq[:, it, h * V : (h + 1) * V],
                        ps_o[:, it * 65 : it * 65 + V], AF.Identity,
                        bias=0.0, scale=rcp[:, it : it + 1],
                    )
            for it in range(NIT):
                nc.scalar.dma_start(
                    oat[:, m, it * 128 : (it + 1) * 128],
                    oatq[:, it, m * 128 : (m + 1) * 128],
                    transpose=True,
                )

        emit_qpair(0)
        emit_qpair(1)
        emit_scores(0)
        emit_qpair(2)
        emit_scores(1)
        emit_scores(2)
        emit_pv_pair(0)
        emit_qpair(3)
        emit_scores(3)
        emit_scores(4)
        emit_pv_pair(1)
        emit_qpair(4)
        emit_scores(5)
        emit_scores(6)
        emit_pv_pair(2)
        emit_qpair(5)
        emit_scores(7)
        emit_scores(8)
        emit_pv_pair(3)
        emit_qpair(6)
        emit_scores(9)
        emit_scores(10)
        emit_pv_pair(4)
        emit_qpair(7)
        emit_scores(11)
        emit_scores(12)
        emit_pv_pair(5)
        emit_scores(13)
        emit_scores(14)
        emit_pv_pair(6)
        emit_scores(15)
        emit_pv_pair(7)
        att_ctx.close()

        with tc.tile_pool(name="psf", bufs=2, space="PSUM") as psfp, tc.tile_pool(
            name="outp", bufs=2
        ) as outp:
            for it in range(NIT):
                for n in range(4):
                    psf = psfp.tile([128, 512], f32, tag="psf")
                    for k in range(8):
                        nc.tensor.matmul(
                            psf[:],
                            lhsT=oat[:, k, it * 128 : (it + 1) * 128],
                            rhs=wo_sb[:, k, n * 512 : (n + 1) * 512],
                            start=(k == 0),
                            stop=(k == 7),
                        )
                    ob = outp.tile([128, 512], f32, tag="ob")
                    nc.vector.tensor_add(
                        ob[:], psf[:], bo_sb[:, n * 512 : (n + 1) * 512]
                    )
                    nc.sync.dma_start(
                        out_d[it * 128 : (it + 1) * 128, n * 512 : (n + 1) * 512],
                        ob[:],
                    )

        if DEBUG:
            nc.sync.dma_start(dbg["lat"][:], latbf[:])
            nc.sync.dma_start(dbg["lat4"][:], lat4[:])
            nc.sync.dma_start(dbg["q"][:], qT[:])
            nc.sync.dma_start(dbg["kf"][:], kf[:])
            nc.sync.dma_start(dbg["v"][:], v65[:])
            nc.sync.dma_start(dbg["oatq"][:], oatq[:])
            nc.sync.dma_start(dbg["oat"][:], oat[:])

    if not bool(int(os.environ.get("BASSNOSPLIT", "0"))):
        _split_multi_waits(nc, mybir)
    nc.finalize()
    return nc


def _split_multi_waits(nc, mybir):
    seq_ok = (mybir.InstEventSemaphore,)
    n = 0
    for fn in nc.m.functions:
        for blk in fn.blocks:
            out = []
            for inst in blk.instructions:
                si = inst.sync_info
                if si is not None and len(si.on_wait) > 1 and not isinstance(inst, seq_ok):
                    if isinstance(inst, mybir.InstDMACopy) and inst.engine not in (
                        mybir.EngineType.SP,
                        mybir.EngineType.Activation,
                    ):
                        raise AssertionError(
                            f"DMA {inst.name} has {len(si.on_wait)} waits; "
                            "restructure so DMAs carry at most one"
                        )
                    for w in si.on_wait[:-1]:
                        n += 1
                        out.append(
                            mybir.InstEventSemaphore(
                                name=f"I-wsplit-{n}",
                                engine=inst.engine,
                                ins=[],
                                outs=[],
                                sync_info=mybir.SyncInfo(on_wait=[w], on_update=[]),
                            )
                        )
                    inst.sync_info = mybir.SyncInfo(
                        on_wait=[si.on_wait[-1]], on_update=si.on_update
                    )
                out.append(inst)
            blk.instructions = out
    return n


def prep_inputs(
    hidden_states, cos, sin, Wq, bq, Wo, bo, Wkva, bkva, w_kc, w_vc, sinks
):
    f16 = np.float16
    hs = np.asarray(hidden_states, np.float32)[0]
    cos = np.asarray(cos, np.float32)[0]
    sin = np.asarray(sin, np.float32)[0]

    def sbuf_pack(a):
        kf_, f_ = a.shape
        k_ = kf_ // 128
        return np.ascontiguousarray(
            a.reshape(k_, 128, f_).transpose(1, 0, 2).reshape(128, k_ * f_)
        )

    wq_kmaj = sbuf_pack(np.asarray(Wq, np.float32).T)
    wqT = np.ascontiguousarray(
        wq_kmaj.reshape(128, 16, 8, 256).transpose(0, 2, 1, 3).reshape(128, -1)
    ).astype(f16)
    wkvaT = sbuf_pack(np.asarray(Wkva, np.float32).T).astype(f16)
    wkc_p = sbuf_pack(
        np.asarray(w_kc, np.float32).transpose(2, 0, 1).reshape(R, H * NOPE)
    ).astype(f16)
    wvc_p = sbuf_pack(
        np.asarray(w_vc, np.float32).transpose(1, 0, 2).reshape(R, H * V)
    ).astype(f16)
    woT = sbuf_pack(np.asarray(Wo, np.float32).T).astype(f16)

    bq_t = np.ascontiguousarray(np.asarray(bq, np.float32).reshape(16, 128).T)
    bkva_pad = np.zeros(640, np.float32)
    bkva_pad[: R + ROPE] = np.asarray(bkva, np.float32)
    bkva_t = np.ascontiguousarray(bkva_pad.reshape(5, 128).T)
    bo_b = np.ascontiguousarray(
        np.broadcast_to(np.asarray(bo, np.float32), (128, HID))
    ).astype(f16)
    esink_b = np.ascontiguousarray(
        np.broadcast_to(np.exp(np.asarray(sinks, np.float32))[None, :], (128, H))
    )
    pp = np.arange(128)[:, None]
    ii = np.arange(128)[None, :]
    mUL = np.zeros((128, 2, 128), np.float32)
    mUL[:, 0, :] = (ii < pp).astype(np.float32)
    mUL[:, 1, :] = (pp <= ii).astype(np.float32)

    hs_pad = np.zeros((SW + S, HID), np.float32)
    hs_pad[SW:] = hs

    shared = dict(
        wqT=wqT, wkvaT=wkvaT, wkc=wkc_p, wvc=wvc_p, woT=woT,
        bq=bq_t, bkva=bkva_t, bo=bo_b, esink=esink_b,
        maskUL=mUL.astype(f16),
    )

    in_maps = []
    for c in range(NCORES):
        g0 = c * Q
        hsT_c = sbuf_pack(np.ascontiguousarray(hs_pad[g0 : g0 + KH].T)).astype(f16)

        cq = cos[g0 : g0 + Q]
        sq = sin[g0 : g0 + Q]
        tqc = np.zeros((128, Q), np.float32)
        tqs = np.zeros((128, Q), np.float32)
        tqc[64:96] = cq[:, 0:32].T
        tqc[96:128] = cq[:, 32:64].T
        tqs[64:96] = sq[:, 0:32].T
        tqs[96:128] = sq[:, 32:64].T

        kpos = np.clip(np.arange(g0 - SW, g0 + Q), 0, None)
        ck = cos[kpos]
        sk = sin[kpos]
        tkk = np.zeros((64, 2, KH), np.float32)
        tkk[0:32, 0] = ck[:, 0:32].T
        tkk[32:64, 0] = ck[:, 32:64].T
        tkk[0:32, 1] = sk[:, 0:32].T
        tkk[32:64, 1] = sk[:, 32:64].T

        jg = (g0 - SW) + np.arange(KH)
        jvalid = (jg.reshape(NJB, 128).T >= 0).astype(np.float32)

        in_maps.append(
            dict(
                shared,
                hsT=hsT_c,
                trigq_cos=tqc.astype(f16),
                trigq_sin=tqs.astype(f16),
                trigk=tkk.astype(f16),
                jvalid=jvalid.astype(f16),
            )
        )
    return in_maps


def get_program():
    if "nc" not in _CACHE:
        _CACHE["nc"] = _build_program()
    return _CACHE["nc"]


def run(in_maps, **kw):
    from concourse.bass_utils import run_bass_kernel_spmd

    nc = get_program()
    return run_bass_kernel_spmd(nc, in_maps, list(range(NCORES)), **kw)


def kernel(**inputs):
    in_maps = prep_inputs(**inputs)
    res = run(in_maps)
    out = np.concatenate([res.results[c]["out"] for c in range(NCORES)], axis=0)
    return out.reshape(B, S, HID).astype(np.float32)
